# revision 6
# baseline (speedup 1.0000x reference)
"""Trainium2 Bass kernel for nn_MultiHeadAttention_46162308498209.

Data-parallel over batch: core b computes the full MHA pipeline for batch
sample b.  All matmuls run in fp16 with fp32 PSUM accumulation.  The
config-dependent attn_arrange scatter is folded into a per-core gathered
output weight W_eff on the host, so the device program is dense and
identical across cores (SPMD).

Device-side dataflow (per core, S=1024, D=1024, H=8, DK=128):
  qT = (Wq x^T)            [D, S]   (lhsT=WqT slice, rhs=xqT tile)
  kT likewise; v = x Wv^T  [S, D]   natural layout
  per head h, per i-block of 512:
    scoresT[j,i] = kT_h^T q_h       (8 j-chunks of 128)
    expT = exp(scoresT / sqrt(DK))  (ScalarE, scale folded into ACT)
    den[1,i]  += ones^T expT        (ones-matmul, PSUM accumulate)
    ctxT[d,i] += v_chunk^T expT     (PSUM accumulate)
    packedT = ctxT * bcast(1/den)   (K=1 broadcast matmul + DVE mult)
  out = packedT^T W_eff + bo        [S, D]
"""

import math
import sys

for _p in ("/opt/trn_rl_repo",):
    if _p not in sys.path:
        sys.path.insert(0, _p)

import numpy as np

H = 8
DK = 128
D = H * DK
B = 8
S = 1024
D_LIST = (32, 64, 96, 128)

P = 128          # partition tile
NB = 512         # moving-dim block (one PSUM bank of fp32)
KC = D // P      # contraction tiles (8)
SOFTMAX_SCALE = 1.0 / math.sqrt(DK)

_COMPILED = None  # compiled Bacc module cache (one compile per process)
_last_in_maps = None


def _build_kernel():
    import concourse.tile as tile
    import concourse.mybir as mybir
    from concourse import bacc

    f32 = mybir.dt.float32
    f16 = mybir.dt.float16
    AF = mybir.ActivationFunctionType

    nc = bacc.Bacc("TRN2", target_bir_lowering=False, debug=False, num_devices=B)

    def din(name, shape, dt=None):
        return nc.dram_tensor(name, shape, dt or f16, kind="ExternalInput").ap()

    xqT = din("xqT", [D, S])
    xkT = din("xkT", [D, S])
    xvT = din("xvT", [D, S])
    wqT = din("wqT", [D, D])
    wkT = din("wkT", [D, D])
    wvT = din("wvT", [D, D])
    weff = din("weff", [D, D])
    bq2 = din("bq2", [P, KC], f32)
    bk2 = din("bk2", [P, KC], f32)
    bv1 = din("bv1", [1, D])
    bo1 = din("bo1", [1, D])
    out = nc.dram_tensor("out", [S, D], f32, kind="ExternalOutput").ap()

    with tile.TileContext(nc) as tc:
        with (
            tc.tile_pool(name="consts", bufs=1) as cpool,
            tc.tile_pool(name="persist", bufs=1) as pp,
        ):
            ones_col = cpool.tile([P, 1], f16, tag="ones_col", name="ones_col")
            nc.vector.memset(ones_col[:], 1.0)
            ones_row = cpool.tile([1, P], f16, tag="ones_row", name="ones_row")
            nc.vector.memset(ones_row[:], 1.0)

            bq_sb = cpool.tile([P, KC], f32, tag="bq", name="bq_sb")
            nc.sync.dma_start(bq_sb[:], bq2[:])
            bk_sb = cpool.tile([P, KC], f32, tag="bk", name="bk_sb")
            nc.sync.dma_start(bk_sb[:], bk2[:])
            bv_sb = cpool.tile([1, D], f16, tag="bv", name="bv_sb")
            nc.sync.dma_start(bv_sb[:], bv1[:])
            bo_sb = cpool.tile([1, D], f16, tag="bo", name="bo_sb")
            nc.sync.dma_start(bo_sb[:], bo1[:])

            # ---------- q/k projections: qT, kT [D, S] as 8 tiles [128, S]
            # stationary = W^T slice [c-chunk, m-chunk] (streamed [128,128]),
            # moving = x^T tile slice [c-chunk, s-block] (x resident).
            def proj_T(x_dram, w_dram, bias_sb, out_tag):
                out_tiles = []
                with (
                    tc.tile_pool(name=f"{out_tag}_x", bufs=1) as xpool,
                    tc.tile_pool(name=f"{out_tag}_w", bufs=12) as wpool,
                    tc.tile_pool(name=f"{out_tag}_ps", bufs=4, space="PSUM") as pspool,
                ):
                    x_tiles = []
                    for c in range(KC):
                        t = xpool.tile([P, S], f16, tag=f"x{c}", name=f"{out_tag}_x{c}")
                        nc.sync.dma_start(t[:], x_dram[c * P : (c + 1) * P, :])
                        x_tiles.append(t)
                    for m in range(KC):
                        o = pp.tile([P, S], f16, tag=f"{out_tag}{m}",
                                    name=f"{out_tag}{m}")
                        out_tiles.append(o)
                        wts = []
                        for c in range(KC):
                            wt = wpool.tile([P, P], f16, tag="w", name="w_slice")
                            nc.sync.dma_start(
                                wt[:],
                                w_dram[c * P : (c + 1) * P, m * P : (m + 1) * P],
                            )
                            wts.append(wt)
                        pss = [
                            pspool.tile([P, NB], f32, tag="ps", name="proj_ps")
                            for _ in range(S // NB)
                        ]
                        for c in range(KC):
                            for sb in range(S // NB):
                                nc.tensor.matmul(
                                    pss[sb][:],
                                    (wts[c][:]),
                                    (x_tiles[c][:, sb * NB : (sb + 1) * NB]),
                                    start=(c == 0),
                                    stop=(c == KC - 1),
                                )
                        for sb in range(S // NB):
                            nc.scalar.activation(
                                o[:, sb * NB : (sb + 1) * NB],
                                pss[sb][:],
                                AF.Identity,
                                bias=bias_sb[:, m : m + 1],
                            )
                return out_tiles

            qT = proj_T(xqT, wqT, bq_sb, "qT")
            kT = proj_T(xkT, wkT, bk_sb, "kT")

            # ---------- v projection: natural layout, 8 s-chunk tiles [128, D]
            # stationary = x^T slice [c-chunk, s-chunk], moving = W^T slice
            # [c-chunk, d-block] (streamed [128,512]).
            v_tiles = [
                pp.tile([P, D], f16, tag=f"v{sc}", name=f"v{sc}")
                for sc in range(KC)
            ]
            with (
                tc.tile_pool(name="v_x", bufs=1) as xpool,
                tc.tile_pool(name="v_w", bufs=10) as wpool,
                tc.tile_pool(name="v_ps", bufs=4, space="PSUM") as pspool,
            ):
                xv_tiles = []
                for c in range(KC):
                    t = xpool.tile([P, S], f16, tag=f"x{c}", name=f"xv{c}")
                    nc.sync.dma_start(t[:], xvT[c * P : (c + 1) * P, :])
                    xv_tiles.append(t)
                for db in range(D // NB):
                    wts = []
                    for c in range(KC):
                        wt = wpool.tile([P, NB], f16, tag="w", name="wv_slice")
                        nc.sync.dma_start(
                            wt[:],
                            wvT[c * P : (c + 1) * P, db * NB : (db + 1) * NB],
                        )
                        wts.append(wt)
                    for sc in range(KC):
                        ps = pspool.tile([P, NB], f32, tag="ps", name="v_ps")
                        for c in range(KC):
                            nc.tensor.matmul(
                                ps[:],
                                (xv_tiles[c][:, sc * P : (sc + 1) * P]),
                                (wts[c][:]),
                                start=(c == 0),
                                stop=False,
                            )
                        nc.tensor.matmul(
                            ps[:],
                            (ones_row[:]),
                            (bv_sb[:, db * NB : (db + 1) * NB]),
                            start=False,
                            stop=True,
                        )
                        nc.vector.tensor_copy(
                            v_tiles[sc][:, db * NB : (db + 1) * NB], ps[:]
                        )

            # ---------- attention (per head, per i-block of 512)
            packedT = [
                pp.tile([P, S], f16, tag=f"packedT{h}", name=f"packedT{h}")
                for h in range(H)
            ]
            with (
                tc.tile_pool(name="att_sb", bufs=1) as att,
                tc.tile_pool(name="att_ps", bufs=3, space="PSUM") as psA,
                tc.tile_pool(name="att_acc", bufs=2, space="PSUM") as psAcc,
                tc.tile_pool(name="att_sm", bufs=1, space="PSUM") as psS,
            ):
                for h in range(H):
                    for ib in range(S // NB):
                        isl = slice(ib * NB, (ib + 1) * NB)
                        ctx_ps = psAcc.tile([P, NB], f32, tag="ctx", name="ctx_ps")
                        den_ps = psS.tile([1, NB], f32, tag="den", name="den_ps")
                        for jc in range(KC):
                            sc_ps = psA.tile([P, NB], f32, tag="sc", name="sc_ps")
                            nc.tensor.matmul(
                                sc_ps[:],
                                (kT[h][:, jc * P : (jc + 1) * P]),
                                (qT[h][:, isl]),
                                start=True,
                                stop=True,
                            )
                            ex = att.tile([P, NB], f16, tag="expt", bufs=10,
                                          name="expt")
                            nc.scalar.activation(
                                ex[:], sc_ps[:], AF.Exp, scale=SOFTMAX_SCALE
                            )
                            nc.tensor.matmul(
                                den_ps[:],
                                (ones_col[:]),
                                (ex[:]),
                                start=(jc == 0),
                                stop=(jc == KC - 1),
                            )
                            nc.tensor.matmul(
                                ctx_ps[:],
                                (v_tiles[jc][:, h * P : (h + 1) * P]),
                                (ex[:]),
                                start=(jc == 0),
                                stop=(jc == KC - 1),
                            )
                        recip = att.tile([1, NB], f16, tag="recip", bufs=2,
                                         name="recip")
                        with nc.allow_low_precision(
                            reason="softmax denominators are O(1e3) and fp16 "
                            "reciprocal keeps ~1e-3 rel, within tolerance"
                        ):
                            nc.vector.reciprocal(recip[:], den_ps[:])
                        bc_ps = psS.tile([P, NB], f32, tag="bcast", name="bc_ps")
                        nc.tensor.matmul(
                            bc_ps[:], (ones_row[:]), (recip[:]),
                            start=True, stop=True,
                        )
                        bc_sb = att.tile([P, NB], f32, tag="bcsb", bufs=2,
                                         name="bc_sb")
                        nc.scalar.copy(bc_sb[:], bc_ps[:])
                        nc.vector.tensor_mul(
                            packedT[h][:, isl], ctx_ps[:], bc_sb[:]
                        )

            # ---------- output projection: out[s, o] = packedT^T W_eff + bo
            with (
                tc.tile_pool(name="op_sb", bufs=1) as op,
                tc.tile_pool(name="op_ps", bufs=4, space="PSUM") as pspool,
            ):
                for ob in range(D // NB):
                    wts = []
                    for cp in range(KC):
                        wt = op.tile([P, NB], f16, tag="weff", bufs=10,
                                     name="weff_slice")
                        nc.sync.dma_start(
                            wt[:],
                            weff[cp * P : (cp + 1) * P, ob * NB : (ob + 1) * NB],
                        )
                        wts.append(wt)
                    for sc in range(KC):
                        ps = pspool.tile([P, NB], f32, tag="ps", name="op_ps")
                        for cp in range(KC):
                            nc.tensor.matmul(
                                ps[:],
                                (packedT[cp][:, sc * P : (sc + 1) * P]),
                                (wts[cp][:]),
                                start=(cp == 0),
                                stop=False,
                            )
                        nc.tensor.matmul(
                            ps[:],
                            (ones_row[:]),
                            (bo_sb[:, ob * NB : (ob + 1) * NB]),
                            start=False,
                            stop=True,
                        )
                        o_sb = op.tile([P, NB], f32, tag="osb", bufs=4,
                                       name="o_sb")
                        nc.vector.tensor_copy(o_sb[:], ps[:])
                        nc.sync.dma_start(
                            out[sc * P : (sc + 1) * P, ob * NB : (ob + 1) * NB],
                            o_sb[:],
                        )

    nc.compile()
    return nc


def _get_nc():
    global _COMPILED
    if _COMPILED is None:
        _COMPILED = _build_kernel()
    return _COMPILED


def kernel(query, key, value, config_idx, Wq, bq, Wk, bk, Wv, bv, Wo, bo,
           **_unused):
    from concourse.bass_utils import run_bass_kernel_spmd

    nc = _get_nc()

    query = np.asarray(query, np.float32)
    key = np.asarray(key, np.float32)
    value = np.asarray(value, np.float32)
    Wq = np.asarray(Wq, np.float32)
    Wk = np.asarray(Wk, np.float32)
    Wv = np.asarray(Wv, np.float32)
    Wo = np.asarray(Wo, np.float32)

    wqT = np.ascontiguousarray(Wq.T).astype(np.float16)
    wkT = np.ascontiguousarray(Wk.T).astype(np.float16)
    wvT = np.ascontiguousarray(Wv.T).astype(np.float16)
    bq2 = np.ascontiguousarray(np.asarray(bq, np.float32).reshape(KC, P).T)
    bk2 = np.ascontiguousarray(np.asarray(bk, np.float32).reshape(KC, P).T)
    bv1 = np.asarray(bv, np.float16).reshape(1, D)
    bo1 = np.asarray(bo, np.float16).reshape(1, D)

    in_maps = []
    for b in range(B):
        d = D_LIST[int(config_idx[b])]
        # fold attn_arrange packing into the output weight:
        # out[s,o] = sum_h sum_{t<d} ctx[s,h,t] * Wo[o, h*d+t]
        weff = np.zeros((D, D), np.float16)
        for h in range(H):
            weff[h * DK : h * DK + d, :] = Wo[:, h * d : h * d + d].T.astype(np.float16)
        in_maps.append(
            {
                "xqT": np.ascontiguousarray(query[b].T).astype(np.float16),
                "xkT": np.ascontiguousarray(key[b].T).astype(np.float16),
                "xvT": np.ascontiguousarray(value[b].T).astype(np.float16),
                "wqT": wqT,
                "wkT": wkT,
                "wvT": wvT,
                "weff": weff,
                "bq2": bq2,
                "bk2": bk2,
                "bv1": bv1,
                "bo1": bo1,
            }
        )

    global _last_in_maps
    _last_in_maps = in_maps
    res = run_bass_kernel_spmd(nc, in_maps, core_ids=list(range(B)))
    return np.stack([res.results[i]["out"] for i in range(B)], axis=0)


# revision 7
# speedup vs baseline: 1.9835x; 1.9835x over previous
"""Trainium2 Bass kernel for nn_MultiHeadAttention_46162308498209.

Data-parallel over batch: core b computes the full MHA pipeline for batch
sample b.  All matmuls run in fp16 with fp32 PSUM accumulation.  The
config-dependent attn_arrange scatter is folded into a per-core gathered
output weight W_eff on the host, so the device program is dense and
identical across cores (SPMD).

Device-side dataflow (per core, S=1024, D=1024, H=8, DK=128):
  qT = (Wq x^T)            [D, S]   (lhsT=packed W slice, rhs=xT tile)
  kT likewise; v = x Wv^T  [S, D]   natural layout
  per head h (both 512-wide i-blocks together, sharing stationaries):
    scoresT[j,i] = kT_h^T q_h       (8 j-chunks of 128)
    expT = exp(scoresT / sqrt(DK))  (ScalarE, scale folded into ACT)
    den[1,i]  += ones^T expT        (ones-matmul, PSUM accumulate)
    ctxT[d,i] += v_chunk^T expT     (PSUM accumulate)
    packedT = ctxT * bcast(1/den)   (K=1 broadcast matmul + DVE mult)
  out = packedT^T W_eff + bo        [S, D]

Weights for the q/k projections are host-packed to [m, p, c*128+j] so each
m-chunk's stationaries arrive as one contiguous [128, 1024] DMA.
"""

import math
import sys

for _p in ("/opt/trn_rl_repo",):
    if _p not in sys.path:
        sys.path.insert(0, _p)

import numpy as np

H = 8
DK = 128
D = H * DK
B = 8
S = 1024
D_LIST = (32, 64, 96, 128)

P = 128          # partition tile
NB = 512         # moving-dim block (one PSUM bank of fp32)
KC = D // P      # contraction tiles (8)
SOFTMAX_SCALE = 1.0 / math.sqrt(DK)

_COMPILED = None  # compiled Bacc module cache (one compile per process)
_last_in_maps = None


def _build_kernel():
    import concourse.tile as tile
    import concourse.mybir as mybir
    from concourse import bacc

    f32 = mybir.dt.float32
    f16 = mybir.dt.float16
    AF = mybir.ActivationFunctionType

    nc = bacc.Bacc("TRN2", target_bir_lowering=False, debug=False, num_devices=B)

    def din(name, shape, dt=None):
        return nc.dram_tensor(name, shape, dt or f16, kind="ExternalInput").ap()

    xqT = din("xqT", [D, S])
    xkT = din("xkT", [D, S])
    xvT = din("xvT", [D, S])
    wqP = din("wqP", [KC, P, D])   # [m, p, c*128+j] packed stationaries
    wkP = din("wkP", [KC, P, D])
    wvT = din("wvT", [D, D])
    weff = din("weff", [D, D])
    bq2 = din("bq2", [P, KC], f32)
    bk2 = din("bk2", [P, KC], f32)
    bv1 = din("bv1", [1, D])
    bo1 = din("bo1", [1, D])
    out = nc.dram_tensor("out", [S, D], f32, kind="ExternalOutput").ap()

    with tile.TileContext(nc) as tc:
        with (
            tc.tile_pool(name="consts", bufs=1) as cpool,
            tc.tile_pool(name="persist", bufs=1) as pp,
        ):
            ones_col = cpool.tile([P, 1], f16, tag="ones_col", name="ones_col")
            nc.vector.memset(ones_col[:], 1.0)
            ones_row = cpool.tile([1, P], f16, tag="ones_row", name="ones_row")
            nc.vector.memset(ones_row[:], 1.0)

            bq_sb = cpool.tile([P, KC], f32, tag="bq", name="bq_sb")
            nc.sync.dma_start(bq_sb[:], bq2[:])
            bk_sb = cpool.tile([P, KC], f32, tag="bk", name="bk_sb")
            nc.sync.dma_start(bk_sb[:], bk2[:])
            bv_sb = cpool.tile([1, D], f16, tag="bv", name="bv_sb")
            nc.sync.dma_start(bv_sb[:], bv1[:])
            bo_sb = cpool.tile([1, D], f16, tag="bo", name="bo_sb")
            nc.sync.dma_start(bo_sb[:], bo1[:])

            # ---------- q/k projections: qT, kT [D, S] as 8 tiles [128, S]
            # stationary = packed W slice [128, 128] (one [128,1024] DMA per
            # m-chunk), moving = x^T tile slice [c-chunk, s-block].
            def proj_T(x_dram, w_dram, bias_sb, out_tag):
                out_tiles = []
                with (
                    tc.tile_pool(name=f"{out_tag}_x", bufs=1) as xpool,
                    tc.tile_pool(name=f"{out_tag}_w", bufs=3) as wpool,
                    tc.tile_pool(name=f"{out_tag}_ps", bufs=4, space="PSUM") as pspool,
                ):
                    x_tiles = []
                    for c in range(KC):
                        t = xpool.tile([P, S], f16, tag=f"x{c}", name=f"{out_tag}_x{c}")
                        nc.sync.dma_start(t[:], x_dram[c * P : (c + 1) * P, :])
                        x_tiles.append(t)
                    for m in range(KC):
                        o = pp.tile([P, S], f16, tag=f"{out_tag}{m}",
                                    name=f"{out_tag}{m}")
                        out_tiles.append(o)
                        wt = wpool.tile([P, D], f16, tag="w", name="w_m")
                        nc.sync.dma_start(wt[:], w_dram[m])
                        pss = [
                            pspool.tile([P, NB], f32, tag="ps", name="proj_ps")
                            for _ in range(S // NB)
                        ]
                        for c in range(KC):
                            for sb in range(S // NB):
                                nc.tensor.matmul(
                                    pss[sb][:],
                                    wt[:, c * P : (c + 1) * P],
                                    x_tiles[c][:, sb * NB : (sb + 1) * NB],
                                    start=(c == 0),
                                    stop=(c == KC - 1),
                                )
                        for sb in range(S // NB):
                            nc.scalar.activation(
                                o[:, sb * NB : (sb + 1) * NB],
                                pss[sb][:],
                                AF.Identity,
                                bias=bias_sb[:, m : m + 1],
                            )
                return out_tiles

            qT = proj_T(xqT, wqP, bq_sb, "qT")
            kT = proj_T(xkT, wkP, bk_sb, "kT")

            # ---------- v projection: natural layout, 8 s-chunk tiles [128, D]
            # stationary = x^T slice [c-chunk, s-chunk] (used for both
            # d-blocks back-to-back), moving = W^T slice [c-chunk, d-block].
            v_tiles = [
                pp.tile([P, D], f16, tag=f"v{sc}", name=f"v{sc}")
                for sc in range(KC)
            ]
            with (
                tc.tile_pool(name="v_x", bufs=1) as xpool,
                tc.tile_pool(name="v_w", bufs=1) as wpool,
                tc.tile_pool(name="v_ps", bufs=4, space="PSUM") as pspool,
            ):
                xv_tiles = []
                for c in range(KC):
                    t = xpool.tile([P, S], f16, tag=f"x{c}", name=f"xv{c}")
                    nc.sync.dma_start(t[:], xvT[c * P : (c + 1) * P, :])
                    xv_tiles.append(t)
                wv_tiles = []
                for c in range(KC):
                    wt = wpool.tile([P, D], f16, tag=f"wv{c}", name=f"wv{c}")
                    nc.sync.dma_start(wt[:], wvT[c * P : (c + 1) * P, :])
                    wv_tiles.append(wt)
                for sc in range(KC):
                    pss = [
                        pspool.tile([P, NB], f32, tag="ps", name="v_ps")
                        for _ in range(D // NB)
                    ]
                    for c in range(KC):
                        for db in range(D // NB):
                            nc.tensor.matmul(
                                pss[db][:],
                                xv_tiles[c][:, sc * P : (sc + 1) * P],
                                wv_tiles[c][:, db * NB : (db + 1) * NB],
                                start=(c == 0),
                                stop=False,
                            )
                    for db in range(D // NB):
                        nc.tensor.matmul(
                            pss[db][:],
                            ones_row[:],
                            bv_sb[:, db * NB : (db + 1) * NB],
                            start=False,
                            stop=True,
                        )
                        nc.vector.tensor_copy(
                            v_tiles[sc][:, db * NB : (db + 1) * NB], pss[db][:]
                        )

            # ---------- attention: per head, both 512-wide i-blocks together
            # so kT / v stationaries serve two matmuls each.
            packedT = [
                pp.tile([P, S], f16, tag=f"packedT{h}", name=f"packedT{h}")
                for h in range(H)
            ]
            NIB = S // NB  # 2
            with (
                tc.tile_pool(name="att_sb", bufs=1) as att,
                tc.tile_pool(name="att_ps", bufs=3, space="PSUM") as psA,
                tc.tile_pool(name="att_acc", bufs=2, space="PSUM") as psAcc,
                tc.tile_pool(name="att_sm", bufs=1, space="PSUM") as psS,
            ):
                for h in range(H):
                    ctx_ps = [
                        psAcc.tile([P, NB], f32, tag="ctx", name="ctx_ps")
                        for _ in range(NIB)
                    ]
                    den_ps = [
                        psS.tile([1, NB], f32, tag=f"den{ib}", name="den_ps")
                        for ib in range(NIB)
                    ]
                    for jc in range(KC):
                        exs = []
                        for ib in range(NIB):
                            sc_ps = psA.tile([P, NB], f32, tag="sc", name="sc_ps")
                            nc.tensor.matmul(
                                sc_ps[:],
                                kT[h][:, jc * P : (jc + 1) * P],
                                qT[h][:, ib * NB : (ib + 1) * NB],
                                start=True,
                                stop=True,
                            )
                            ex = att.tile([P, NB], f16, tag="expt", bufs=8,
                                          name="expt")
                            nc.scalar.activation(
                                ex[:], sc_ps[:], AF.Exp, scale=SOFTMAX_SCALE
                            )
                            exs.append(ex)
                        for ib in range(NIB):
                            nc.tensor.matmul(
                                den_ps[ib][:],
                                ones_col[:],
                                exs[ib][:],
                                start=(jc == 0),
                                stop=(jc == KC - 1),
                            )
                        for ib in range(NIB):
                            nc.tensor.matmul(
                                ctx_ps[ib][:],
                                v_tiles[jc][:, h * P : (h + 1) * P],
                                exs[ib][:],
                                start=(jc == 0),
                                stop=(jc == KC - 1),
                            )
                    for ib in range(NIB):
                        recip = att.tile([1, NB], f16, tag="recip", bufs=2,
                                         name="recip")
                        with nc.allow_low_precision(
                            reason="softmax denominators are O(1e3); fp16 "
                            "reciprocal keeps ~1e-3 rel, within tolerance"
                        ):
                            nc.vector.reciprocal(recip[:], den_ps[ib][:])
                        bc_ps = psS.tile([P, NB], f32, tag="bcast", name="bc_ps")
                        nc.tensor.matmul(
                            bc_ps[:], ones_row[:], recip[:], start=True, stop=True
                        )
                        bc_sb = att.tile([P, NB], f32, tag="bcsb", bufs=2,
                                         name="bc_sb")
                        nc.scalar.copy(bc_sb[:], bc_ps[:])
                        nc.vector.tensor_mul(
                            packedT[h][:, ib * NB : (ib + 1) * NB],
                            ctx_ps[ib][:],
                            bc_sb[:],
                        )

            # ---------- output projection: out[s, o] = packedT^T W_eff + bo
            # stationary = packedT slice (reused for both o-blocks).
            with (
                tc.tile_pool(name="op_sb", bufs=1) as op,
                tc.tile_pool(name="op_ps", bufs=4, space="PSUM") as pspool,
            ):
                weff_tiles = []
                for cp in range(KC):
                    wt = op.tile([P, D], f16, tag=f"weff{cp}", name=f"weff{cp}")
                    nc.sync.dma_start(wt[:], weff[cp * P : (cp + 1) * P, :])
                    weff_tiles.append(wt)
                for sc in range(KC):
                    pss = [
                        pspool.tile([P, NB], f32, tag="ps", name="op_ps")
                        for _ in range(D // NB)
                    ]
                    for cp in range(KC):
                        for ob in range(D // NB):
                            nc.tensor.matmul(
                                pss[ob][:],
                                packedT[cp][:, sc * P : (sc + 1) * P],
                                weff_tiles[cp][:, ob * NB : (ob + 1) * NB],
                                start=(cp == 0),
                                stop=False,
                            )
                    for ob in range(D // NB):
                        nc.tensor.matmul(
                            pss[ob][:],
                            ones_row[:],
                            bo_sb[:, ob * NB : (ob + 1) * NB],
                            start=False,
                            stop=True,
                        )
                        o_sb = op.tile([P, NB], f32, tag="osb", bufs=4,
                                       name="o_sb")
                        nc.vector.tensor_copy(o_sb[:], pss[ob][:])
                        nc.sync.dma_start(
                            out[sc * P : (sc + 1) * P, ob * NB : (ob + 1) * NB],
                            o_sb[:],
                        )

    nc.compile()
    return nc


def _get_nc():
    global _COMPILED
    if _COMPILED is None:
        _COMPILED = _build_kernel()
    return _COMPILED


def _pack_w(W):
    # [m, p, c*128+j] = W[m*128+j, c*128+p]
    return np.ascontiguousarray(
        np.transpose(np.asarray(W, np.float32).reshape(KC, P, KC, P), (0, 3, 2, 1))
        .reshape(KC, P, D)
        .astype(np.float16)
    )


def kernel(query, key, value, config_idx, Wq, bq, Wk, bk, Wv, bv, Wo, bo,
           **_unused):
    from concourse.bass_utils import run_bass_kernel_spmd

    nc = _get_nc()

    query = np.asarray(query, np.float32)
    key = np.asarray(key, np.float32)
    value = np.asarray(value, np.float32)
    Wo = np.asarray(Wo, np.float32)

    wqP = _pack_w(Wq)
    wkP = _pack_w(Wk)
    wvT = np.ascontiguousarray(np.asarray(Wv, np.float32).T).astype(np.float16)
    bq2 = np.ascontiguousarray(np.asarray(bq, np.float32).reshape(KC, P).T)
    bk2 = np.ascontiguousarray(np.asarray(bk, np.float32).reshape(KC, P).T)
    bv1 = np.asarray(bv, np.float16).reshape(1, D)
    bo1 = np.asarray(bo, np.float16).reshape(1, D)

    in_maps = []
    for b in range(B):
        d = D_LIST[int(config_idx[b])]
        # fold attn_arrange packing into the output weight:
        # out[s,o] = sum_h sum_{t<d} ctx[s,h,t] * Wo[o, h*d+t]
        weff = np.zeros((D, D), np.float16)
        for h in range(H):
            weff[h * DK : h * DK + d, :] = Wo[:, h * d : h * d + d].T.astype(
                np.float16
            )
        in_maps.append(
            {
                "xqT": np.ascontiguousarray(query[b].T).astype(np.float16),
                "xkT": np.ascontiguousarray(key[b].T).astype(np.float16),
                "xvT": np.ascontiguousarray(value[b].T).astype(np.float16),
                "wqP": wqP,
                "wkP": wkP,
                "wvT": wvT,
                "weff": weff,
                "bq2": bq2,
                "bk2": bk2,
                "bv1": bv1,
                "bo1": bo1,
            }
        )

    global _last_in_maps
    _last_in_maps = in_maps
    res = run_bass_kernel_spmd(nc, in_maps, core_ids=list(range(B)))
    return np.stack([res.results[i]["out"] for i in range(B)], axis=0)
